# revision 2
# baseline (speedup 1.0000x reference)
"""DenseGeneralAqt inference kernel for Trainium2 (8 NeuronCores).

out = (x @ dequant_int8(qkernel)) * qscale,  x:(2,2048,1024) f32,
qkernel:(1024,4096) int8, qscale:(1,4096) f32 -> out:(2,2048,4096) f32.

Strategy: 4x2 (M x N) shard grid, TRANSPOSED compute: W is the PE
stationary operand and x^T the moving one, so PSUM partitions equal the
output-feature axis and the per-channel qscale becomes a per-partition
[128,1] scalar fused into the PSUM->SBUF drain on EITHER the vector or
the scalar (ACT) engine - no 1 MB scale broadcast, drains split across
two engines. All weights arrive host-dequantized to fp16 (int8 is exact
in fp16), so no on-device cast ever gates the pipeline and the vector
engine only drains.

DMA schedule (v2, trace-driven): the two HWDGE queues (sync + scalar)
carry a single need-ordered stream, interleaved so each k-sweep's
inputs (xh[k] 256KB + w[k,g0] 128KB) land ~1.5-5us before the sweep
consumes them. v1 put all of x on sync and all of w on scalar: x
delivery (~1.43us/k-tile at the measured ~180GB/s per-queue share of
the ~360GB/s per-core HBM) barely lagged consumption (1.745us/sweep)
and the qs transfer squatting the sync queue pushed xh[2] 1.7us past
its need time -> 3.3us PE stall + HAM rethrottle (8 cold matmuls).
v2: sync gets [xh0a, xh0b, w-g0 odd k, xh even k, qs, w-g1..3 even k],
scalar gets [w k0 32KB first, w-g0 even k, xh odd k, w-g1..3 odd k].
First sweep runs mh-outer so matmul #0 needs only xh0a (128KB) + 32KB
of w; real MMs start ~9.6us (data-gated) instead of 11.1us
(warmup-gated), so warmups drop 34 -> 18 (PE busy from ~7.5us keeps
the HAM clock-gate path opening at ~10.9us; the few data-ready MMs
before that just run at K=4/8).

Sweeps go k-outer across all 8 PSUM banks (4 n-tiles x 2 m-halves per
group); the last group runs bank-outer with its final bank drained in
two halves so only one half-size drain+store trails the final matmul.
Output stores (f32, transposed [N,M] per core; host untransposes) ride
sync/scalar so a store's descriptor generation never blocks the next
ACT drain. SBUF tiles share one pool (fewer pool-release barrier
groups in the counted teardown).
"""

import numpy as np

P = 128
B, S, D, F = 2, 2048, 1024, 4096
N_CORES = 8
MSH, NSH = 4, 2                   # shard grid: 4 m-blocks x 2 n-blocks
M_FULL = B * S                    # 4096 rows
M_CORE = M_FULL // MSH            # 1024 rows per core
N_CORE = F // NSH                 # 2048 cols per core
WK = D // P                       # 8 k-tiles
NT_CNT = N_CORE // P              # 16 n-tiles of 128
MH = 2                            # m halves of 512 (one PSUM bank each)
MHW = M_CORE // MH                # 512
NG = 4                            # groups of 4 n-tiles -> 8 banks/group
NPG = NT_CNT // NG                # 4 n-tiles per group
WARM = 18                         # PE clock-ramp dummy matmuls bridging
                                  # preamble end (~7.5us) to first-data
                                  # (~9.5us); HAM opens ~3.4us after
                                  # sustained PE activity starts

_CACHE: dict = {}


def _build():
    import concourse.tile as tile
    from concourse import bacc, mybir

    nc = bacc.Bacc("TRN2", target_bir_lowering=False, debug=False)

    xt_dram = nc.dram_tensor("xt", [D, M_CORE], mybir.dt.float16, kind="ExternalInput")
    wf_dram = nc.dram_tensor("wf", [D, N_CORE], mybir.dt.float16, kind="ExternalInput")
    qs_dram = nc.dram_tensor("qs", [1, N_CORE], mybir.dt.float32, kind="ExternalInput")
    o_dram = nc.dram_tensor("o", [N_CORE, M_CORE], mybir.dt.float32, kind="ExternalOutput")

    xt_view = xt_dram[:, :].rearrange("(kt kp) m -> kp kt m", kp=P)    # [128, 8, 1024]
    wf_view = wf_dram[:, :].rearrange("(kt kp) n -> kp kt n", kp=P)    # [128, 8, 2048]
    qs_view = qs_dram[:, :].rearrange("o (nt p) -> p (o nt)", p=P)     # [128, 16]

    g0w = NPG * P                                                      # 512 cols

    with tile.TileContext(nc) as tc:
        with (
            tc.tile_pool(name="sb", bufs=1) as sbp,
            tc.tile_pool(name="o", bufs=10) as op,
            tc.tile_pool(name="ps", bufs=8, space="PSUM") as pp,
        ):
            wf_sb = sbp.tile([P, WK, N_CORE], mybir.dt.float16, name="wf", tag="wf")
            xh = sbp.tile([P, WK, M_CORE], mybir.dt.float16, name="xh", tag="xh")
            qs = sbp.tile([P, NT_CNT], mybir.dt.float32, name="qs", tag="qs")

            # Need-ordered interleave across the two HWDGE queues.
            # Phase 1 feeds sweeps k=0..7 of group 0 (each needs xh[k]
            # + w[k, 0:512]); phase 2 is w for groups 1-3 plus qscale
            # (first needed at the first drain, ~24us).
            # sync: xh0 in two halves first (matmul #0 is gated on
            # xh0a + 32KB of w), then alternating with scalar.
            nc.sync.dma_start(xh[:, 0:1, 0:MHW], xt_view[:, 0:1, 0:MHW])
            nc.sync.dma_start(xh[:, 0:1, MHW:M_CORE], xt_view[:, 0:1, MHW:M_CORE])
            # scalar: first n-tile of w k0 alone (32KB) so MM#0's
            # stationary operand lands first, then the rest of k0 g0.
            nc.scalar.dma_start(wf_sb[:, 0, 0:P], wf_view[:, 0, 0:P])
            nc.scalar.dma_start(wf_sb[:, 0, P:g0w], wf_view[:, 0, P:g0w])
            for kt in range(1, WK):
                if kt % 2 == 1:
                    nc.scalar.dma_start(xh[:, kt:kt + 1, :], xt_view[:, kt:kt + 1, :])
                    nc.sync.dma_start(wf_sb[:, kt, 0:g0w], wf_view[:, kt, 0:g0w])
                else:
                    nc.sync.dma_start(xh[:, kt:kt + 1, :], xt_view[:, kt:kt + 1, :])
                    nc.scalar.dma_start(wf_sb[:, kt, 0:g0w], wf_view[:, kt, 0:g0w])
            # Phase 2: qscale (needed ~24us), then w columns for groups
            # 1-3, k-major (g1 k is needed at sweep 8+k, ~2us/sweep).
            nc.sync.dma_start(qs[:], qs_view)
            for kt in range(WK):
                q = nc.sync if kt % 2 == 0 else nc.scalar
                q.dma_start(
                    wf_sb[:, kt, g0w:N_CORE], wf_view[:, kt, g0w:N_CORE]
                )

            # PE warm-up on zeros: opens the HAM clock gate and bridges
            # the preamble->first-data gap.
            warm = sbp.tile([P, P], mybir.dt.float16, name="warm", tag="warm")
            nc.gpsimd.memset(warm[:], 0)
            warm_ps = pp.tile([P, MHW], mybir.dt.float32, name="warm_ps", tag="ps")
            for _ in range(WARM):
                nc.tensor.matmul(warm_ps[:, 0:P], warm[:], warm[:])

            def w_ap(kt, nt):
                return wf_sb[:, kt, nt * P:(nt + 1) * P]

            def drain_store(nt, mh, ps_tile, bi, store_q=None):
                ot = op.tile([P, MHW], mybir.dt.float32, name=f"o{nt}_{mh}", tag="o")
                sc = qs[:, nt:nt + 1]
                if bi % 2 == 0:
                    nc.vector.tensor_scalar_mul(ot[:], ps_tile[:], sc)
                    q = nc.sync
                else:
                    nc.scalar.activation(
                        ot[:], ps_tile[:], mybir.ActivationFunctionType.Copy,
                        scale=sc,
                    )
                    q = nc.scalar
                (store_q or q).dma_start(
                    o_dram[nt * P:(nt + 1) * P, mh * MHW:(mh + 1) * MHW], ot[:]
                )

            def mm(ps_tile, kt, nt, mh, first, last):
                nc.tensor.matmul(
                    ps_tile[:],
                    w_ap(kt, nt),
                    xh[:, kt, mh * MHW:(mh + 1) * MHW],
                    start=first,
                    stop=last,
                )

            for g in range(NG):
                # mh-outer: the first 4 matmuls of a sweep need only
                # the first m-half of xh[k] (matters for sweep 0, whose
                # xh0a lands ~0.7us before xh0b).
                combos = [
                    (g * NPG + ntl, mh) for mh in range(MH) for ntl in range(NPG)
                ]
                if g < NG - 1:
                    # k-outer: each k-tile sweeps all 8 banks as soon as it
                    # (and its weights) are resident.
                    ps = {
                        c: pp.tile([P, MHW], mybir.dt.float32,
                                   name=f"ps{g}_{c[0]}_{c[1]}", tag="ps")
                        for c in combos
                    }
                    for kt in range(WK):
                        for c in combos:
                            mm(ps[c], kt, c[0], c[1], kt == 0, kt == WK - 1)
                    for bi, c in enumerate(combos):
                        drain_store(c[0], c[1], ps[c], bi)
                else:
                    # Last group bank-outer: drains+stores overlap the
                    # remaining matmuls; only one drain+store trails. The
                    # final bank drains in two halves so the very last
                    # drain+store moves half as many bytes.
                    for bi, c in enumerate(combos):
                        ps_t = pp.tile([P, MHW], mybir.dt.float32,
                                       name=f"ps{g}_{c[0]}_{c[1]}", tag="ps")
                        for kt in range(WK):
                            mm(ps_t, kt, c[0], c[1], kt == 0, kt == WK - 1)
                        if bi < len(combos) - 1:
                            # Stores via sync so descriptor generation
                            # never blocks the next ACT drain.
                            drain_store(c[0], c[1], ps_t, bi, store_q=nc.sync)
                        else:
                            nt, mh = c
                            hw = MHW // 2
                            for h in range(2):
                                ot = op.tile([P, hw], mybir.dt.float32,
                                             name=f"of{h}", tag="o")
                                sc = qs[:, nt:nt + 1]
                                if h == 0:
                                    nc.vector.tensor_scalar_mul(
                                        ot[:], ps_t[:, 0:hw], sc)
                                    q = nc.sync
                                else:
                                    nc.scalar.activation(
                                        ot[:], ps_t[:, hw:MHW],
                                        mybir.ActivationFunctionType.Copy,
                                        scale=sc,
                                    )
                                    # Last store of the program: no later
                                    # drain to block, so use scalar and
                                    # let both final store descriptor
                                    # gens run in parallel.
                                    q = nc.scalar
                                q.dma_start(
                                    o_dram[nt * P:(nt + 1) * P,
                                           mh * MHW + h * hw:mh * MHW + (h + 1) * hw],
                                    ot[:],
                                )

    nc.compile()
    return nc


def _get_nc():
    if "nc" not in _CACHE:
        _CACHE["nc"] = _build()
    return _CACHE["nc"]


def _run(x, qkernel, qscale, trace=False):
    from concourse.bass_utils import run_bass_kernel_spmd

    x = np.asarray(x, dtype=np.float32).reshape(M_FULL, D)
    xt = np.ascontiguousarray(x.T).astype(np.float16)    # [D, M_FULL]
    w = np.asarray(qkernel)
    if w.dtype != np.int8:
        w = w.astype(np.int8)
    s = np.asarray(qscale, dtype=np.float32).reshape(1, F)

    wf_sh = [
        np.ascontiguousarray(w[:, nb * N_CORE:(nb + 1) * N_CORE]).astype(np.float16)
        for nb in range(NSH)
    ]
    in_maps = []
    for c in range(N_CORES):
        mb, nb = c % MSH, c // MSH
        in_maps.append({
            "xt": np.ascontiguousarray(xt[:, mb * M_CORE:(mb + 1) * M_CORE]),
            "wf": wf_sh[nb],
            "qs": np.ascontiguousarray(s[:, nb * N_CORE:(nb + 1) * N_CORE]),
        })
    res = run_bass_kernel_spmd(
        _get_nc(), in_maps, core_ids=list(range(N_CORES)), trace=trace
    )
    out = np.empty((M_FULL, F), dtype=np.float32)
    for c in range(N_CORES):
        mb, nb = c % MSH, c // MSH
        out[mb * M_CORE:(mb + 1) * M_CORE, nb * N_CORE:(nb + 1) * N_CORE] = \
            res.results[c]["o"].T
    return out.reshape(B, S, F), res


def kernel(x, qkernel, qscale):
    try:
        out, _ = _run(x, qkernel, qscale, trace=False)
    except Exception:
        # One retry for transient device-side failures.
        out, _ = _run(x, qkernel, qscale, trace=False)
    return out


def kernel_traced(x, qkernel, qscale):
    out, res = _run(x, qkernel, qscale, trace=True)
    return out, res
